# revision 1
# baseline (speedup 1.0000x reference)
import numpy as np
import ml_dtypes

# GCN 3-layer Trainium2 kernel — 8 cores, single launch, on-device gather.
#
# Aggregate-first GCN: act_next = relu((A_hat @ act) @ W + b).
# dst-node rows sharded 8 ways (12500/core, padded to 12544).
# Per core, per layer:
#   * dma_gather (gpsimd SWDGE 'mlp' firmware) fetches per-edge source rows
#     from a replicated bf16 activation table in DRAM. int16 gather indices
#     force 4 source-row ranges of 25088 rows.
#   * DVE scales gathered rows by the per-edge GCN norm (pads carry norm=0),
#     builds one-hot scatter blocks S from dst-local ids vs an iota ramp.
#   * TensorE: psum[k,n] += xg_chunk^T @ S_chunk over a window's chunks
#     (scatter-add), then aggT @ W dense transform per 128-row window.
#   * DVE adds bias (+relu for layers 1-2); shard written to DRAM; AllGather
#     collective replicates activations for the next layer.
# Edge chunks are padded to the max count over all 8 cores per (window,range)
# so one SPMD program serves every core (only the data differs).

N = 100000
F = 128
NC = 8
NPC = N // NC
WIN = 128
NW = (NPC + WIN - 1) // WIN
NPAD = NW * WIN
NFULL = NC * NPAD
NR = 4
BW = 7                      # windows per gather block
MAXG = 8192                 # max gather indices per SWDGE instruction (carveout)
REPLICATE_IDX = True

_cache = {}
EXEC_NS = []


def _range_size():
    return (NFULL + NR - 1) // NR


def _preprocess(edge_index):
    RANGE = _range_size()
    NB = NW // BW
    src = np.asarray(edge_index[0], dtype=np.int64)
    dst = np.asarray(edge_index[1], dtype=np.int64)
    loop = np.arange(N, dtype=np.int64)
    src = np.concatenate([src, loop])
    dst = np.concatenate([dst, loop])
    deg = np.bincount(dst, minlength=N).astype(np.float32)
    dinv = np.where(deg > 0, 1.0 / np.sqrt(deg), 0.0).astype(np.float32)
    norm = (dinv[src] * dinv[dst]).astype(np.float32)

    srcp = (src // NPC) * NPAD + (src % NPC)      # padded table row
    core = dst // NPC
    dl = dst - core * NPC
    w = dl // WIN
    b = w // BW
    r = srcp // RANGE
    srcl = (srcp % RANGE).astype(np.int64)
    dwin = (dl % WIN).astype(np.float32)

    order = np.lexsort((w, r, b, core))
    core_s, w_s, r_s = core[order], w[order], r[order]
    srcl_s, dwin_s, norm_s = srcl[order], dwin[order], norm[order]

    gid = (core_s * NW + w_s) * NR + r_s
    cnt = np.bincount(gid, minlength=NC * NW * NR).reshape(NC, NW, NR)
    kwr = (cnt.max(axis=0) + 127) // 128          # [NW, NR]
    KMAX = int(kwr.sum(axis=1).max())
    NCH = int(kwr.sum())
    R_total = NCH * 128

    group_off = np.zeros((NW, NR), dtype=np.int64)
    off = 0
    per_block_off = []
    per_block_chunks = []
    gather_list = []
    for bb in range(NB):
        per_block_off.append(off)
        blk = 0
        glist = []
        for rr in range(NR):
            first = off + blk
            n_idx = 0
            for ww in range(bb * BW, (bb + 1) * BW):
                group_off[ww, rr] = off + blk
                blk += int(kwr[ww, rr])
                n_idx += int(kwr[ww, rr]) * 128
            # split into <=MAXG-index SWDGE instructions
            cur = first
            rem = n_idx
            while rem > 0:
                take = min(rem, MAXG)
                glist.append((rr, cur, take))
                cur += take // 128
                rem -= take
        gather_list.append(glist)
        per_block_chunks.append(blk)
        off += blk
    assert off == NCH

    win_segs = [
        [(int(group_off[ww, rr]), int(kwr[ww, rr]))
         for rr in range(NR) if kwr[ww, rr] > 0]
        for ww in range(NW)
    ]

    # rank of each edge within its (core,window,range) group; groups are
    # contiguous in the sorted order but not in gid order, so derive starts
    # from run boundaries of the sorted gid sequence.
    change = np.r_[True, gid[1:] != gid[:-1]]
    group_first = np.flatnonzero(change)
    sizes = np.diff(np.r_[group_first, len(gid)])
    rank = np.arange(len(gid)) - np.repeat(group_first, sizes)
    slot = group_off[w_s, r_s] * 128 + rank

    idx_all = np.zeros((NC, R_total), dtype=np.int16)
    dstl_all = np.zeros((NC, R_total), dtype=np.float32)
    norm_all = np.zeros((NC, R_total), dtype=np.float32)
    idx_all[core_s, slot] = srcl_s.astype(np.int16)
    dstl_all[core_s, slot] = dwin_s
    norm_all[core_s, slot] = norm_s

    bf = ml_dtypes.bfloat16
    idxw = np.zeros((NC, 128, R_total // 16), dtype=np.int16)
    wrapped = idx_all.reshape(NC, R_total // 16, 16).transpose(0, 2, 1)
    for k in range(8 if REPLICATE_IDX else 1):
        idxw[:, k * 16:(k + 1) * 16, :] = wrapped
    dstl_sb = np.ascontiguousarray(
        dstl_all.reshape(NC, NCH, 128).transpose(0, 2, 1)).astype(bf)
    norm_sb = np.ascontiguousarray(
        norm_all.reshape(NC, NCH, 128).transpose(0, 2, 1)).astype(bf)

    meta = dict(KMAX=KMAX, NCH=NCH, R_total=R_total, NB=NB,
                per_block_chunks=per_block_chunks,
                per_block_off=per_block_off,
                gather_list=gather_list, win_segs=win_segs)
    return meta, idxw, dstl_sb, norm_sb


def _build_program(meta):
    import concourse.bass as bass
    import concourse.mybir as mybir
    from concourse import bacc
    from concourse.tile import TileContext

    RANGE = _range_size()
    KMAX = meta["KMAX"]
    NCH = meta["NCH"]
    R_total = meta["R_total"]
    NB = meta["NB"]
    per_block_chunks = meta["per_block_chunks"]
    per_block_off = meta["per_block_off"]
    gather_list = meta["gather_list"]
    win_segs = meta["win_segs"]
    CBMAX = max(per_block_chunks)
    ICOLS = R_total // 16

    nc = bacc.Bacc(None, target_bir_lowering=False, num_devices=NC)
    bf16 = mybir.dt.bfloat16
    i16 = mybir.dt.int16
    f32 = mybir.dt.float32

    xsh_d = nc.dram_tensor("xsh", [NPAD, F], bf16, kind="ExternalInput")
    idx_d = nc.dram_tensor("idx", [128, ICOLS], i16, kind="ExternalInput")
    dstl_d = nc.dram_tensor("dstl", [128, NCH], bf16, kind="ExternalInput")
    norm_d = nc.dram_tensor("normv", [128, NCH], bf16, kind="ExternalInput")
    W_d = nc.dram_tensor("W", [128, 3 * F], bf16, kind="ExternalInput")
    bias_d = nc.dram_tensor("bias", [128, 3 * F], f32, kind="ExternalInput")
    iota_d = nc.dram_tensor("iota", [128, KMAX * F], bf16, kind="ExternalInput")
    out_d = nc.dram_tensor("out", [NPAD, F], f32, kind="ExternalOutput")
    act_a = nc.dram_tensor("act_a", [NFULL, F], bf16)
    act_b = nc.dram_tensor("act_b", [NFULL, F], bf16)
    shard = nc.dram_tensor("shard", [NPAD, F], bf16)

    rg = [list(range(NC))]

    with TileContext(nc) as tc:
        with (
            tc.tile_pool(name="res", bufs=1) as res,
            tc.tile_pool(name="xgp", bufs=2) as xgp,
            tc.tile_pool(name="swp", bufs=2) as swp,
            tc.tile_pool(name="smal", bufs=3) as smal,
            tc.tile_pool(name="psp", bufs=2, space="PSUM") as psp,
        ):
            gat_reg = nc.gpsimd.alloc_register()
            idx_s = res.tile([128, ICOLS], i16)
            dstl_s = res.tile([128, NCH], bf16)
            norm_s = res.tile([128, NCH], bf16)
            W_s = res.tile([128, 3 * F], bf16)
            bias_s = res.tile([128, 3 * F], f32)
            iota_s = res.tile([128, KMAX * F], bf16)
            nc.sync.dma_start(out=idx_s[:, :], in_=idx_d[:, :])
            nc.sync.dma_start(out=dstl_s[:, :], in_=dstl_d[:, :])
            nc.sync.dma_start(out=norm_s[:, :], in_=norm_d[:, :])
            nc.sync.dma_start(out=W_s[:, :], in_=W_d[:, :])
            nc.sync.dma_start(out=bias_s[:, :], in_=bias_d[:, :])
            nc.sync.dma_start(out=iota_s[:, :], in_=iota_d[:, :])

            nc.sync.dma_start(out=shard[:, :], in_=xsh_d[:, :])
            nc.gpsimd.collective_compute(
                "AllGather", mybir.AluOpType.bypass, replica_groups=rg,
                ins=[shard.ap().opt()], outs=[act_a.ap().opt()],
            )

            for l in range(3):
                src_t = act_a if l % 2 == 0 else act_b
                for b in range(NB):
                    off0 = per_block_off[b]
                    cb = per_block_chunks[b]
                    xg_t = xgp.tile([128, CBMAX * F], bf16, tag="xg")
                    for (rr, ch0, n_idx) in gather_list[b]:
                        c0 = ch0 - off0
                        nc.gpsimd.reg_mov(gat_reg, n_idx)
                        nc.gpsimd.dma_gather(
                            out_ap=xg_t[:, c0 * F:(c0 + n_idx // 128) * F]
                            .rearrange("p (c f) -> p c f", f=F),
                            in_ap=src_t[rr * RANGE:(rr + 1) * RANGE, :],
                            idxs_ap=idx_s[:, ch0 * 8:ch0 * 8 + n_idx // 16],
                            num_idxs=n_idx,
                            num_idxs_reg=gat_reg,
                            elem_size=F,
                            single_packet=False,
                        )
                    nc.vector.tensor_tensor(
                        out=xg_t[:, :cb * F],
                        in0=xg_t[:, :cb * F],
                        in1=norm_s[:, off0:off0 + cb].to_broadcast([128, cb, F]),
                        op=mybir.AluOpType.mult,
                    )
                    for wi in range(BW):
                        w = b * BW + wi
                        segs = win_segs[w]
                        kw = sum(k for _, k in segs)
                        S_t = swp.tile([128, KMAX * F], bf16, tag="S")
                        pos = 0
                        for (chpos, k) in segs:
                            nc.vector.tensor_tensor(
                                out=S_t[:, pos * F:(pos + k) * F],
                                in0=dstl_s[:, chpos:chpos + k]
                                .to_broadcast([128, k, F]),
                                in1=iota_s[:, :k * F],
                                op=mybir.AluOpType.is_equal,
                            )
                            pos += k
                        ps1 = psp.tile([128, F], f32, tag="ps1")
                        done = 0
                        pos = 0
                        for (chpos, k) in segs:
                            for c in range(k):
                                nc.tensor.matmul(
                                    ps1[:, :],
                                    xg_t[:, (chpos - off0 + c) * F:
                                         (chpos - off0 + c + 1) * F],
                                    S_t[:, (pos + c) * F:(pos + c + 1) * F],
                                    start=(done == 0),
                                    stop=(done == kw - 1),
                                )
                                done += 1
                            pos += k
                        aggT_t = smal.tile([128, F], bf16, tag="aggT")
                        nc.vector.tensor_copy(out=aggT_t[:, :], in_=ps1[:, :])
                        ps2 = psp.tile([128, F], f32, tag="ps2")
                        nc.tensor.matmul(
                            ps2[:, :], aggT_t[:, :],
                            W_s[:, l * F:(l + 1) * F],
                            start=True, stop=True,
                        )
                        if l < 2:
                            at = smal.tile([128, F], bf16, tag="at")
                            nc.vector.tensor_add(
                                out=at[:, :], in0=ps2[:, :],
                                in1=bias_s[:, l * F:(l + 1) * F])
                            nc.vector.tensor_scalar_max(at[:, :], at[:, :], 0.0)
                            nc.sync.dma_start(
                                out=shard[w * WIN:(w + 1) * WIN, :],
                                in_=at[:, :])
                        else:
                            ot = smal.tile([128, F], f32, tag="ot")
                            nc.vector.tensor_add(
                                out=ot[:, :], in0=ps2[:, :],
                                in1=bias_s[:, l * F:(l + 1) * F])
                            nc.sync.dma_start(
                                out=out_d[w * WIN:(w + 1) * WIN, :],
                                in_=ot[:, :])
                if l < 2:
                    dst_t = act_b if l % 2 == 0 else act_a
                    nc.gpsimd.collective_compute(
                        "AllGather", mybir.AluOpType.bypass, replica_groups=rg,
                        ins=[shard.ap().opt()], outs=[dst_t.ap().opt()],
                    )
    nc.compile()
    return nc


def _prepare(edge_index):
    key = "prep"
    if key not in _cache:
        meta, idxw, dstl_sb, norm_sb = _preprocess(edge_index)
        prog = _build_program(meta)
        _cache[key] = (meta, idxw, dstl_sb, norm_sb, prog)
    return _cache[key]


def kernel(x, edge_index, W1, b1, W2, b2, W3, b3):
    from concourse.bass_utils import run_bass_kernel_spmd

    bf = ml_dtypes.bfloat16
    meta, idxw, dstl_sb, norm_sb, prog = _prepare(edge_index)
    KMAX = meta["KMAX"]

    x = np.asarray(x, dtype=np.float32)
    xpad = np.zeros((NC, NPAD, F), dtype=bf)
    xpad[:, :NPC, :] = x.reshape(NC, NPC, F).astype(bf)

    Wall = np.stack([np.asarray(Wl, np.float32) for Wl in (W1, W2, W3)], 0)
    Wtile = np.concatenate([Wl.astype(bf) for Wl in Wall], axis=1)  # [128, 384]
    ball = [np.asarray(bl, np.float32) for bl in (b1, b2, b3)]
    btile = np.concatenate(
        [np.broadcast_to(bl[None, :], (128, F)) for bl in ball], axis=1
    ).astype(np.float32)
    iota = np.tile(np.arange(F, dtype=np.float32), KMAX)[None, :]
    iota = np.broadcast_to(iota, (128, KMAX * F)).astype(bf)

    in_maps = []
    for c in range(NC):
        in_maps.append({
            "xsh": np.ascontiguousarray(xpad[c]),
            "idx": np.ascontiguousarray(idxw[c]),
            "dstl": np.ascontiguousarray(dstl_sb[c]),
            "normv": np.ascontiguousarray(norm_sb[c]),
            "W": np.ascontiguousarray(Wtile),
            "bias": np.ascontiguousarray(btile),
            "iota": np.ascontiguousarray(iota),
        })
    import time
    t0 = time.perf_counter_ns()
    res = run_bass_kernel_spmd(prog, in_maps, list(range(NC)))
    t1 = time.perf_counter_ns()
    EXEC_NS.append(res.exec_time_ns if getattr(res, "exec_time_ns", None)
                   else t1 - t0)
    outs = []
    for c in range(NC):
        r = res.results[c]
        if isinstance(r, dict):
            r = r["out"]
        elif isinstance(r, (list, tuple)):
            r = r[0]
        outs.append(np.asarray(r)[:NPC])
    return np.concatenate(outs, axis=0).astype(np.float32)



# revision 6
# speedup vs baseline: 4.3049x; 4.3049x over previous
import numpy as np
import ml_dtypes

# GCN 3-layer Trainium2 kernel — 8 cores, single launch, scatter-add design.
#
# norm factorization: norm = dinv[src]*dinv[dst], so the activation table is
# pre-scaled by dinv (t[i] = dinv[i]*h[i]) and aggregates are post-scaled by
# dinv[dst] after the dense transform ((D*A)@W = D*(A@W)). No per-edge norm.
#
# Per layer, per core (dst rows sharded 8 ways, 12500/core padded to 12544):
#   * dma_gather (gpsimd SWDGE) fetches per-edge source rows from the
#     replicated f16 table in DRAM (int16 idx -> 4 source ranges of 25088).
#   * dma_scatter_add accumulates rows into an f16 DRAM buffer by local dst
#     row. The DMA's RMW drops duplicate-row updates within one instruction,
#     so edges are bucketed by rank-within-(core,range,dst): every scatter
#     instruction touches each dst row at most once; buckets serialize via
#     tile WAW deps. Pad slots scatter to a dump row.
#   * Per 128-row window: PE-transpose the aggregate, dense matmul with W,
#     then dinv-scale + bias (+relu, + dinv pre-scale for the next table).
#   * AllGather replicates the next table across cores.
# Slot counts are padded to the max over the 8 cores per (range, rank-bucket)
# so one SPMD program serves every core (only the data differs).

N = 100000
F = 128
NC = 8
NPC = N // NC               # 12500
WIN = 128
NW = (NPC + WIN - 1) // WIN  # 98
NPAD = NW * WIN             # 12544
NFULL = NC * NPAD           # 100352
NR = 4
RANGE = NFULL // NR         # 25088
MAXG = 8192                 # max idx per SWDGE gather instruction
MAXS = 4096                 # max idx per scatter (RMW needs 2x descs)
DUMP = NPAD                 # scatter dump row for pad slots
NAGG = NPAD + 128           # agg rows incl. dump block (99*128)
KB = 64                     # rank-bucket cap

_cache = {}
EXEC_NS = []


def _preprocess(edge_index):
    ei = np.asarray(edge_index)
    src = ei[0].astype(np.int32, copy=False)
    dst = ei[1].astype(np.int32, copy=False)
    loop = np.arange(N, dtype=np.int32)
    src = np.concatenate([src, loop])
    dst = np.concatenate([dst, loop])
    E = src.shape[0]

    deg = np.bincount(dst, minlength=N).astype(np.float32)
    dinv = 1.0 / np.sqrt(deg)   # every node has a self loop -> deg >= 1

    srcp = (src // NPC) * NPAD + (src % NPC)
    r = (srcp // RANGE).astype(np.int32)
    srcl = (srcp % RANGE).astype(np.int16)
    core = dst // NPC
    dl = (dst - core * NPC).astype(np.int16)

    cr = core * NR + r                      # 0..31
    # rank of each edge within its (core, range, dst) group
    key1 = cr * 131072 + dst
    o1 = np.argsort(key1, kind="stable")
    k1s = key1[o1]
    change = np.r_[True, k1s[1:] != k1s[:-1]]
    starts = np.flatnonzero(change)
    sizes = np.diff(np.r_[starts, E])
    rank_s = np.arange(E, dtype=np.int32) - np.repeat(starts, sizes)
    kk = np.empty(E, np.int32)
    kk[o1] = rank_s
    assert kk.max() < KB

    # order by (core, range, rank-bucket, dst)
    key2 = (cr * KB + kk) * 131072 + dst
    o2 = np.argsort(key2, kind="stable")
    core_s = core[o2]
    srcl_s = srcl[o2]
    dl_s = dl[o2]

    bk = cr * KB + kk                       # [0, NC*NR*KB)
    cnt = np.bincount(bk, minlength=NC * NR * KB).reshape(NC, NR * KB)
    mx = cnt.max(axis=0)                    # [NR*KB]
    BS = ((mx + 127) // 128) * 128
    off2 = np.concatenate([[0], np.cumsum(BS)[:-1]]).astype(np.int64)
    NSLOT = int(BS.sum())

    # rank within each (core, range, bucket) group in o2 order
    cntf = cnt.reshape(-1)                  # (core,(r,k)) C-order == key2 order
    startsf = np.cumsum(cntf) - cntf
    rank3 = np.arange(E, dtype=np.int64) - np.repeat(startsf, cntf)
    rk_s = (cr * KB + kk)[o2] % (NR * KB)
    slot = off2[rk_s] + rank3

    gidx = np.zeros((NC, NSLOT), np.int16)
    sidx = np.full((NC, NSLOT), DUMP, np.int16)
    gidx[core_s, slot] = srcl_s
    sidx[core_s, slot] = dl_s
    g16 = np.ascontiguousarray(
        gidx.reshape(NC, NSLOT // 16, 16).transpose(0, 2, 1))
    s16 = np.ascontiguousarray(
        sidx.reshape(NC, NSLOT // 16, 16).transpose(0, 2, 1))

    plan = []
    for rr in range(NR):
        for k in range(KB):
            b = int(BS[rr * KB + k])
            if b == 0:
                continue
            base = int(off2[rr * KB + k])
            for c0 in range(0, b, MAXG):
                plan.append((rr, base + c0, min(MAXG, b - c0)))

    f16 = np.float16
    dinv_pad = np.zeros((NC, NPAD), np.float32)
    dinv_pad[:, :NPC] = dinv.reshape(NC, NPC)
    dinv_sb = np.ascontiguousarray(
        dinv_pad.reshape(NC, NW, WIN).transpose(0, 2, 1)).astype(f16)

    meta = dict(NSLOT=NSLOT, plan=plan)
    return meta, g16, s16, dinv_sb, dinv


def _build_program(meta):
    import concourse.mybir as mybir
    from concourse import bacc
    from concourse.tile import TileContext

    NSLOT = meta["NSLOT"]
    plan = meta["plan"]
    IC = NSLOT // 16

    nc = bacc.Bacc(None, target_bir_lowering=False, num_devices=NC)
    f16 = mybir.dt.float16
    i16 = mybir.dt.int16
    f32 = mybir.dt.float32

    xsh_d = nc.dram_tensor("xsh", [NPAD, F], f16, kind="ExternalInput")
    gid_d = nc.dram_tensor("gid", [16, IC], i16, kind="ExternalInput")
    sid_d = nc.dram_tensor("sid", [16, IC], i16, kind="ExternalInput")
    dinv_d = nc.dram_tensor("dinv", [128, NW], f16, kind="ExternalInput")
    W_d = nc.dram_tensor("W", [128, 3 * F], f16, kind="ExternalInput")
    brow_d = nc.dram_tensor("brow", [1, 3 * F], f32, kind="ExternalInput")
    out_d = nc.dram_tensor("out", [NPAD, F], f16, kind="ExternalOutput")
    act_a = nc.dram_tensor("act_a", [NFULL, F], f16)
    act_b = nc.dram_tensor("act_b", [NFULL, F], f16)
    agg_d = nc.dram_tensor("agg", [NAGG, F], f16)
    zz_d = nc.dram_tensor("zz", [NAGG, F], f16)
    shard = nc.dram_tensor("shard", [NPAD, F], f16)

    rg = [list(range(NC))]

    with TileContext(nc) as tc:
        with (
            tc.tile_pool(name="res", bufs=1) as res,
            tc.tile_pool(name="gb", bufs=3) as gb,
            tc.tile_pool(name="wp", bufs=3) as wp,
            tc.tile_pool(name="psp", bufs=2, space="PSUM") as psp,
        ):
            gid_s = res.tile([128, IC], i16)
            sid_s = res.tile([128, IC], i16)
            W_s = res.tile([128, 3 * F], f16)
            dinv_s = res.tile([128, NW], f16)
            brow_s = res.tile([1, 3 * F], f32)
            for k in range(8):
                nc.sync.dma_start(out=gid_s[16 * k:16 * (k + 1), :],
                                  in_=gid_d[:, :])
                nc.sync.dma_start(out=sid_s[16 * k:16 * (k + 1), :],
                                  in_=sid_d[:, :])
            nc.sync.dma_start(out=W_s[:, :], in_=W_d[:, :])
            nc.sync.dma_start(out=dinv_s[:, :], in_=dinv_d[:, :])
            nc.sync.dma_start(out=brow_s[:, :], in_=brow_d[:, :])

            # bias broadcast [128, 3F] via ones outer product
            ones_s = res.tile([1, 128], f16)
            nc.vector.memset(ones_s[:, :], 1.0)
            brow_h = res.tile([1, 3 * F], f16)
            nc.vector.tensor_copy(out=brow_h[:, :], in_=brow_s[:, :])
            psB = psp.tile([128, 3 * F], f32, tag="psB")
            nc.tensor.matmul(psB[:, :], ones_s[:, :], brow_h[:, :],
                             start=True, stop=True)
            biasB = res.tile([128, 3 * F], f32)
            nc.vector.tensor_copy(out=biasB[:, :], in_=psB[:, :])

            # identity for PE transpose
            ic_t = res.tile([128, 128], f16)
            ir_t = res.tile([128, 128], f16)
            nc.gpsimd.iota(ic_t[:, :], pattern=[[1, 128]], base=0,
                           channel_multiplier=0,
                           allow_small_or_imprecise_dtypes=True)
            nc.gpsimd.iota(ir_t[:, :], pattern=[[0, 128]], base=0,
                           channel_multiplier=1,
                           allow_small_or_imprecise_dtypes=True)
            ident = res.tile([128, 128], f16)
            nc.vector.tensor_tensor(out=ident[:, :], in0=ic_t[:, :],
                                    in1=ir_t[:, :],
                                    op=mybir.AluOpType.is_equal)

            # zeros source for agg reset
            zero_s = res.tile([128, F], f16)
            nc.vector.memset(zero_s[:, :], 0.0)
            for w in range(NAGG // 128):
                nc.sync.dma_start(out=zz_d[w * 128:(w + 1) * 128, :],
                                  in_=zero_s[:, :])

            nc.sync.dma_start(out=shard[:, :], in_=xsh_d[:, :])
            nc.gpsimd.collective_compute(
                "AllGather", mybir.AluOpType.bypass, replica_groups=rg,
                ins=[shard.ap().opt()], outs=[act_a.ap().opt()],
            )

            for l in range(3):
                tab = act_a if l % 2 == 0 else act_b
                nc.sync.dma_start(out=agg_d[:, :], in_=zz_d[:, :])
                for (rr, s0, n) in plan:
                    cn = n // 128
                    g = gb.tile([128, MAXG // 128, F], f16, tag="g")
                    nc.gpsimd.dma_gather(
                        out_ap=g[:, :cn, :],
                        in_ap=tab[rr * RANGE:(rr + 1) * RANGE, :],
                        idxs_ap=gid_s[:, s0 // 16:(s0 + n) // 16],
                        num_idxs=n,
                        num_idxs_reg=n,
                        elem_size=F,
                        single_packet=False,
                    )
                    for c0 in range(0, n, MAXS):
                        m = min(MAXS, n - c0)
                        nc.gpsimd.dma_scatter_add(
                            agg_d[:, :],
                            g[:, c0 // 128:(c0 + m) // 128, :],
                            sid_s[:, (s0 + c0) // 16:(s0 + c0 + m) // 16],
                            m,
                            m,
                            F,
                        )
                for w in range(NW):
                    a_t = wp.tile([128, F], f16, tag="a")
                    nc.sync.dma_start(out=a_t[:, :],
                                      in_=agg_d[w * 128:(w + 1) * 128, :])
                    tr = psp.tile([128, F], f16, tag="tr")
                    nc.tensor.transpose(tr[:, :], a_t[:, :], ident[:, :])
                    zT = wp.tile([128, F], f16, tag="zT")
                    nc.vector.tensor_copy(out=zT[:, :], in_=tr[:, :])
                    p2 = psp.tile([128, F], f32, tag="p2")
                    nc.tensor.matmul(p2[:, :], zT[:, :],
                                     W_s[:, l * F:(l + 1) * F],
                                     start=True, stop=True)
                    dv = dinv_s[:, w:w + 1].to_broadcast([128, 1, F])
                    e1 = wp.tile([128, F], f32, tag="e1")
                    nc.vector.tensor_tensor(out=e1[:, :], in0=p2[:, :],
                                            in1=dv, op=mybir.AluOpType.mult)
                    if l < 2:
                        nc.vector.tensor_add(out=e1[:, :], in0=e1[:, :],
                                             in1=biasB[:, l * F:(l + 1) * F])
                        nc.vector.tensor_scalar_max(e1[:, :], e1[:, :], 0.0)
                        o_t = wp.tile([128, F], f16, tag="o")
                        nc.vector.tensor_tensor(out=o_t[:, :], in0=e1[:, :],
                                                in1=dv,
                                                op=mybir.AluOpType.mult)
                        nc.sync.dma_start(
                            out=shard[w * WIN:(w + 1) * WIN, :],
                            in_=o_t[:, :])
                    else:
                        o_t = wp.tile([128, F], f16, tag="o")
                        nc.vector.tensor_add(out=o_t[:, :], in0=e1[:, :],
                                             in1=biasB[:, l * F:(l + 1) * F])
                        nc.sync.dma_start(
                            out=out_d[w * WIN:(w + 1) * WIN, :],
                            in_=o_t[:, :])
                if l < 2:
                    dst_t = act_b if l % 2 == 0 else act_a
                    nc.gpsimd.collective_compute(
                        "AllGather", mybir.AluOpType.bypass, replica_groups=rg,
                        ins=[shard.ap().opt()], outs=[dst_t.ap().opt()],
                    )
    nc.compile()
    return nc


def _prepare(edge_index):
    if "prep" not in _cache:
        meta, g16, s16, dinv_sb, dinv = _preprocess(edge_index)
        prog = _build_program(meta)
        _cache["prep"] = (meta, g16, s16, dinv_sb, dinv, prog)
    return _cache["prep"]


def kernel(x, edge_index, W1, b1, W2, b2, W3, b3):
    from concourse.bass_utils import run_bass_kernel_spmd

    f16 = np.float16
    meta, g16, s16, dinv_sb, dinv, prog = _prepare(edge_index)

    x = np.asarray(x, dtype=np.float32)
    xs = x * dinv[:, None]
    xpad = np.zeros((NC, NPAD, F), dtype=f16)
    xpad[:, :NPC, :] = xs.reshape(NC, NPC, F).astype(f16)

    Wtile = np.concatenate(
        [np.asarray(Wl, np.float32).astype(f16) for Wl in (W1, W2, W3)],
        axis=1)
    brow = np.concatenate(
        [np.asarray(bl, np.float32) for bl in (b1, b2, b3)])[None, :]

    in_maps = []
    for c in range(NC):
        in_maps.append({
            "xsh": np.ascontiguousarray(xpad[c]),
            "gid": np.ascontiguousarray(g16[c]),
            "sid": np.ascontiguousarray(s16[c]),
            "dinv": np.ascontiguousarray(dinv_sb[c]),
            "W": np.ascontiguousarray(Wtile),
            "brow": np.ascontiguousarray(brow.astype(np.float32)),
        })
    import time
    t0 = time.perf_counter_ns()
    res = run_bass_kernel_spmd(prog, in_maps, list(range(NC)))
    t1 = time.perf_counter_ns()
    EXEC_NS.append(res.exec_time_ns if getattr(res, "exec_time_ns", None)
                   else t1 - t0)
    outs = []
    for c in range(NC):
        r = res.results[c]
        if isinstance(r, dict):
            r = r["out"]
        elif isinstance(r, (list, tuple)):
            r = r[0]
        outs.append(np.asarray(r)[:NPC])
    return np.concatenate(outs, axis=0).astype(np.float32)
